# revision 1
# baseline (speedup 1.0000x reference)
"""Complex GRU cell on 8 Trainium2 NeuronCores (Bass/Tile).

Strategy
  - Data-parallel: batch 16384 -> 8 cores x 2048; 512x512 weights replicated.
  - Feature-major (transposed) layout on device: host pre-transposes x,h to
    [D, B_local] fp16 and pre-transposes the weights, so the kernel needs no
    on-device transposes and per-feature biases land on SBUF partitions
    (fused into ACT ops).
  - Complex matmul uses the Gauss 3-multiplication trick: with host-prepared
    weight variants Wr, (Wi-Wr), -(Wr+Wi) and input sums (Xr+Xi),
      C = (Xr+Xi)@Wr, A = Xi@(-(Wr+Wi)), B = Xr@(Wi-Wr)
      re = C + A, im = C + B
    i.e. 3 fp16 matmul groups (fp32 PSUM accumulate) + one ACT copy + two DVE
    adds, instead of 4 matmul groups -- 25% less TensorE work.
  - Per batch-chunk of 512: R wave (sigmoid gate), r*h on DVE, Z wave,
    C wave (x part + (r*h) part), polar-tanh via ACT Sqrt/Tanh plus
    1/|c| = exp(-0.5 ln |c|^2) on ACT, final complex blend h + z*(h~ - h).
"""
import sys

for _p in ("/opt/trn_rl_repo",):
    if _p not in sys.path:
        sys.path.insert(0, _p)

import numpy as np
import concourse.bass as bass
import concourse.tile as tile
import concourse.mybir as mybir
from concourse.bass_utils import run_bass_kernel_spmd

F32, F16 = mybir.dt.float32, mybir.dt.float16
AF = mybir.ActivationFunctionType
ALU = mybir.AluOpType

RE, IM, IMN = 0, 1, 2  # weight variant slots: Wr, (Wi-Wr), -(Wr+Wi)
GZ, GR, GH = 0, 1, 2   # gates (z, r, candidate)

N_CORES = 8
B_FULL, D, H = 16384, 512, 512
B_LOCAL = B_FULL // N_CORES
BCHUNK = 512

LAST_RUN_INFO = {}
_CACHE = {}


def _split_waits(nc, maxw=1):
    """walrus here allows 1 sync wait per instruction; hoist extras onto NoOps."""
    for fn in nc.m.functions:
        for bb in fn.blocks:
            out = []
            for inst in list(bb.instructions):
                si = inst.sync_info
                waits = list(si.on_wait) if si is not None else []
                if len(waits) > maxw:
                    extra, keep = waits[:-maxw], waits[-maxw:]
                    k = 0
                    while extra:
                        chunk, extra = extra[:maxw], extra[maxw:]
                        out.append(mybir.InstNoOp(
                            name=f"{inst.name}-wsplit{k}", engine=inst.engine,
                            ins=[], outs=[],
                            sync_info=mybir.SyncInfo(on_wait=chunk, on_update=[])))
                        k += 1
                    inst.sync_info = mybir.SyncInfo(on_wait=keep,
                                                    on_update=list(si.on_update))
                out.append(inst)
            bb.instructions[:] = out
    return nc


def _build(split_for_hw=True):
    NBC = B_LOCAL // BCHUNK
    nc = bass.Bass("TRN2", target_bir_lowering=False, debug=False)

    dram_acts = {}
    for nm in ("xr", "xi", "xs", "hr", "hi", "hs"):
        dram_acts[nm] = nc.dram_tensor(nm, [4, 128, B_LOCAL], F16,
                                       kind="ExternalInput")
    wx = nc.dram_tensor("wx", [3, 3, 4, 128, 512], F16, kind="ExternalInput")
    wh = nc.dram_tensor("wh", [3, 3, 4, 128, 512], F16, kind="ExternalInput")
    bias = nc.dram_tensor("bias", [3, 2, 4, 128], F32, kind="ExternalInput")
    outr = nc.dram_tensor("outr", [512, B_LOCAL], F32, kind="ExternalOutput")
    outi = nc.dram_tensor("outi", [512, B_LOCAL], F32, kind="ExternalOutput")

    with tile.TileContext(nc) as tc:
        with (
            tc.tile_pool(name="wpool", bufs=1) as wpool,
            tc.tile_pool(name="apool", bufs=2) as apool,
            tc.tile_pool(name="rhpool", bufs=1) as rhpool,
            tc.tile_pool(name="zpool", bufs=1) as zpool,
            tc.tile_pool(name="spool", bufs=2) as spool,
            tc.tile_pool(name="opool", bufs=1) as opool,
            tc.tile_pool(name="cpool", bufs=1) as cpool,
            tc.tile_pool(name="pspool", bufs=8, space="PSUM") as pspool,
        ):
            W = {}
            for which, src in (("x", wx), ("h", wh)):
                for g in range(3):
                    for v in range(3):
                        for dt in range(4):
                            t = wpool.tile([128, 512], F16, tag=f"w{which}{g}{v}{dt}")
                            nc.sync.dma_start(t[:], src[g, v, dt])
                            W[(which, g, v, dt)] = t
            BT = {}
            for g in range(3):
                for comp in range(2):
                    for t4 in range(4):
                        t = wpool.tile([128, 1], F32, tag=f"b{g}{comp}{t4}")
                        nc.sync.dma_start(
                            t[:], bias[g, comp, t4].rearrange("(p o) -> p o", o=1))
                        BT[(g, comp, t4)] = t

            def bank_mms(ps, g, v, srcs, t4, first_start, last_stop=True):
                """One Gauss product group accumulated into psum tile ps.
                srcs: list of ("x"|"h", act_tile_dict)."""
                n = len(srcs) * 4
                i = 0
                for which, act in srcs:
                    for dt in range(4):
                        nc.tensor.matmul(
                            ps[:],
                            W[(which, g, v, dt)][:, t4 * 128:(t4 + 1) * 128],
                            act[dt][:],
                            start=(first_start and i == 0),
                            stop=(last_stop and i == n - 1))
                        i += 1

            def gauss_combine(A, Bk, C, tagp):
                """re = C + A, im = C + B via one ACT copy + two DVE adds."""
                csb = spool.tile([128, BCHUNK], F16, tag="csb")
                nc.scalar.activation(csb[:], C[:], AF.Identity)
                pre_r = spool.tile([128, BCHUNK], F16, tag=f"{tagp}r")
                pre_i = spool.tile([128, BCHUNK], F16, tag=f"{tagp}i")
                nc.vector.tensor_tensor(pre_r[:], A[:], csb[:], ALU.add)
                nc.vector.tensor_tensor(pre_i[:], Bk[:], csb[:], ALU.add)
                return pre_r, pre_i

            for bc in range(NBC):
                bsl = slice(bc * BCHUNK, (bc + 1) * BCHUNK)
                act = {}
                for nm in ("xr", "xi", "xs", "hr", "hi", "hs"):
                    d = {}
                    for dt in range(4):
                        t = apool.tile([128, BCHUNK], F16, tag=f"a{nm}{dt}")
                        nc.sync.dma_start(t[:], dram_acts[nm][dt, :, bsl])
                        d[dt] = t
                    act[nm] = d

                # R wave: r = cv_sigmoid(px1 + pr); rh = r*h (+ sum for Gauss)
                rh_re, rh_im, rh_s = {}, {}, {}
                for t4 in range(4):
                    A = pspool.tile([128, BCHUNK], F32, tag="ps")
                    bank_mms(A, GR, IMN, [("x", act["xi"]), ("h", act["hi"])], t4, True)
                    Bk = pspool.tile([128, BCHUNK], F32, tag="ps")
                    bank_mms(Bk, GR, IM, [("x", act["xr"]), ("h", act["hr"])], t4, True)
                    C = pspool.tile([128, BCHUNK], F32, tag="ps")
                    bank_mms(C, GR, RE, [("x", act["xs"]), ("h", act["hs"])], t4, True)
                    pre_r, pre_i = gauss_combine(A, Bk, C, "rp")
                    rr = spool.tile([128, BCHUNK], F16, tag="r0")
                    ri = spool.tile([128, BCHUNK], F16, tag="r1")
                    nc.scalar.activation(rr[:], pre_r[:], AF.Sigmoid,
                                         bias=BT[(GR, 0, t4)][:])
                    nc.scalar.activation(ri[:], pre_i[:], AF.Sigmoid,
                                         bias=BT[(GR, 1, t4)][:])
                    t1 = spool.tile([128, BCHUNK], F16, tag="t1")
                    t2 = spool.tile([128, BCHUNK], F16, tag="t2")
                    nc.vector.tensor_tensor(t1[:], rr[:], act["hr"][t4][:], ALU.mult)
                    nc.vector.tensor_tensor(t2[:], ri[:], act["hi"][t4][:], ALU.mult)
                    rhr = rhpool.tile([128, BCHUNK], F16, tag=f"rhr{t4}")
                    nc.vector.tensor_tensor(rhr[:], t1[:], t2[:], ALU.subtract)
                    t3 = spool.tile([128, BCHUNK], F16, tag="t1")
                    t4b = spool.tile([128, BCHUNK], F16, tag="t2")
                    nc.vector.tensor_tensor(t3[:], rr[:], act["hi"][t4][:], ALU.mult)
                    nc.vector.tensor_tensor(t4b[:], ri[:], act["hr"][t4][:], ALU.mult)
                    rhi = rhpool.tile([128, BCHUNK], F16, tag=f"rhi{t4}")
                    nc.vector.tensor_tensor(rhi[:], t3[:], t4b[:], ALU.add)
                    rhs = rhpool.tile([128, BCHUNK], F16, tag=f"rhs{t4}")
                    nc.vector.tensor_tensor(rhs[:], rhr[:], rhi[:], ALU.add)
                    rh_re[t4], rh_im[t4], rh_s[t4] = rhr, rhi, rhs

                # Z wave: z = cv_sigmoid(px0 + pz)
                z16 = {}
                for t4 in range(4):
                    A = pspool.tile([128, BCHUNK], F32, tag="ps")
                    bank_mms(A, GZ, IMN, [("x", act["xi"]), ("h", act["hi"])], t4, True)
                    Bk = pspool.tile([128, BCHUNK], F32, tag="ps")
                    bank_mms(Bk, GZ, IM, [("x", act["xr"]), ("h", act["hr"])], t4, True)
                    C = pspool.tile([128, BCHUNK], F32, tag="ps")
                    bank_mms(C, GZ, RE, [("x", act["xs"]), ("h", act["hs"])], t4, True)
                    pre_r, pre_i = gauss_combine(A, Bk, C, "zp")
                    for comp, pre in ((0, pre_r), (1, pre_i)):
                        zt = zpool.tile([128, BCHUNK], F16, tag=f"z{t4}{comp}")
                        nc.scalar.activation(zt[:], pre[:], AF.Sigmoid,
                                             bias=BT[(GZ, comp, t4)][:])
                        z16[(t4, comp)] = zt

                # C wave: c = px2 + (r*h)@Wh2^T; h_tilde = polar_tanh(c + b)
                # alpha pass: everything through Ln/Exp (one table-set switch
                # into natural_log_exp; Identity/Square are fillers in every
                # set so they don't thrash the ACT tables)
                cb16, mag16, inv16 = {}, {}, {}
                for pair in ((0, 1), (2, 3)):
                  for t4 in pair:
                     A = pspool.tile([128, BCHUNK], F32, tag="ps")
                     bank_mms(A, GH, IMN, [("x", act["xi"])], t4, True, last_stop=False)
                     Bk = pspool.tile([128, BCHUNK], F32, tag="ps")
                     bank_mms(Bk, GH, IM, [("x", act["xr"])], t4, True, last_stop=False)
                     C = pspool.tile([128, BCHUNK], F32, tag="ps")
                     bank_mms(C, GH, RE, [("x", act["xs"])], t4, True, last_stop=False)
                     bank_mms(A, GH, IMN, [("h", rh_im)], t4, False)
                     bank_mms(Bk, GH, IM, [("h", rh_re)], t4, False)
                     bank_mms(C, GH, RE, [("h", rh_s)], t4, False)
                     pre_r, pre_i = gauss_combine(A, Bk, C, "cp")
                     bre, bim = BT[(GH, 0, t4)], BT[(GH, 1, t4)]
                     cbr = cpool.tile([128, BCHUNK], F16, tag=f"cbr{t4%2}")
                     cbi = cpool.tile([128, BCHUNK], F16, tag=f"cbi{t4%2}")
                     nc.scalar.activation(cbr[:], pre_r[:], AF.Identity, bias=bre[:])
                     nc.scalar.activation(cbi[:], pre_i[:], AF.Identity, bias=bim[:])
                     sre = spool.tile([128, BCHUNK], F16, tag="sre")
                     sim_ = spool.tile([128, BCHUNK], F16, tag="sim")
                     nc.scalar.activation(sre[:], pre_r[:], AF.Square, bias=bre[:])
                     nc.scalar.activation(sim_[:], pre_i[:], AF.Square, bias=bim[:])
                     m2 = spool.tile([128, BCHUNK], F16, tag="m2")
                     nc.vector.tensor_tensor(m2[:], sre[:], sim_[:], ALU.add)
                     # mag = exp(0.5 ln m2), 1/mag = exp(-0.5 ln m2): stays in
                     # the natural_log_exp table set (no sqrt set needed; the
                     # custom-DVE recip breaks this walrus build and DVE
                     # iterative reciprocal is ~8x slower)
                     lnm = spool.tile([128, BCHUNK], F32, tag="lnm")
                     nc.scalar.activation(lnm[:], m2[:], AF.Ln)
                     mag = cpool.tile([128, BCHUNK], F16, tag=f"mag{t4%2}")
                     nc.scalar.activation(mag[:], lnm[:], AF.Exp, scale=0.5)
                     inv = cpool.tile([128, BCHUNK], F16, tag=f"inv{t4%2}")
                     nc.scalar.activation(inv[:], lnm[:], AF.Exp, scale=-0.5)
                     cb16[t4], mag16[t4], inv16[t4] = (cbr, cbi), mag, inv

                  # beta pass: Tanh (sigmoid/tanh table set -- same set the
                  # next chunk's sigmoids use) + blend h_new = h+z*(h_tilde-h)
                  for t4 in pair:
                    cbr, cbi = cb16[t4]
                    th = spool.tile([128, BCHUNK], F16, tag="th")
                    nc.scalar.activation(th[:], mag16[t4][:], AF.Tanh)
                    tf = spool.tile([128, BCHUNK], F16, tag="tf")
                    nc.vector.tensor_tensor(tf[:], th[:], inv16[t4][:], ALU.mult)
                    htr = spool.tile([128, BCHUNK], F16, tag="htr")
                    hti = spool.tile([128, BCHUNK], F16, tag="hti")
                    nc.vector.tensor_tensor(htr[:], tf[:], cbr[:], ALU.mult)
                    nc.vector.tensor_tensor(hti[:], tf[:], cbi[:], ALU.mult)

                    # final: h_new = h + z*(h_tilde - h)
                    dre = spool.tile([128, BCHUNK], F16, tag="dre")
                    dim = spool.tile([128, BCHUNK], F16, tag="dim")
                    nc.vector.tensor_tensor(dre[:], htr[:], act["hr"][t4][:], ALU.subtract)
                    nc.vector.tensor_tensor(dim[:], hti[:], act["hi"][t4][:], ALU.subtract)
                    zr, zi = z16[(t4, 0)], z16[(t4, 1)]
                    u1 = spool.tile([128, BCHUNK], F16, tag="u1")
                    u2 = spool.tile([128, BCHUNK], F16, tag="u2")
                    nc.vector.tensor_tensor(u1[:], zr[:], dre[:], ALU.mult)
                    nc.vector.tensor_tensor(u2[:], zi[:], dim[:], ALU.mult)
                    ere = spool.tile([128, BCHUNK], F16, tag="ere")
                    nc.vector.tensor_tensor(ere[:], u1[:], u2[:], ALU.subtract)
                    u3 = spool.tile([128, BCHUNK], F16, tag="u1")
                    u4 = spool.tile([128, BCHUNK], F16, tag="u2")
                    nc.vector.tensor_tensor(u3[:], zr[:], dim[:], ALU.mult)
                    nc.vector.tensor_tensor(u4[:], zi[:], dre[:], ALU.mult)
                    eim = spool.tile([128, BCHUNK], F16, tag="eim")
                    nc.vector.tensor_tensor(eim[:], u3[:], u4[:], ALU.add)
                    orr = opool.tile([128, BCHUNK], F32, tag="or")
                    oii = opool.tile([128, BCHUNK], F32, tag="oi")
                    nc.vector.tensor_tensor(orr[:], act["hr"][t4][:], ere[:], ALU.add)
                    nc.vector.tensor_tensor(oii[:], act["hi"][t4][:], eim[:], ALU.add)
                    nc.sync.dma_start(outr[t4 * 128:(t4 + 1) * 128, bsl], orr[:])
                    nc.sync.dma_start(outi[t4 * 128:(t4 + 1) * 128, bsl], oii[:])

    if split_for_hw:
        _split_waits(nc)
    return nc


def _prep(inputs):
    x_re, x_im = inputs["x_re"], inputs["x_im"]
    h_re, h_im = inputs["h_re"], inputs["h_im"]

    def actT(a, sl):
        return np.ascontiguousarray(
            a[sl].T.reshape(4, 128, B_LOCAL).astype(np.float16))

    def wvar(Wre, Wim):
        out = np.empty((3, 3, 4, 128, 512), np.float16)
        for g in range(3):
            WreT, WimT = Wre[g].T, Wim[g].T
            out[g, RE] = WreT.reshape(4, 128, 512)
            out[g, IM] = (WimT - WreT).reshape(4, 128, 512)
            out[g, IMN] = (-(WreT + WimT)).reshape(4, 128, 512)
        return out

    wxn = wvar(inputs["Wx_re"], inputs["Wx_im"])
    whn = wvar(inputs["Wh_re"], inputs["Wh_im"])
    bias = np.stack([inputs["bx_re"] + inputs["bh_re"],
                     inputs["bx_im"] + inputs["bh_im"]],
                    axis=1).reshape(3, 2, 4, 128).astype(np.float32)
    x_s = x_re + x_im
    h_s = h_re + h_im

    in_maps = []
    for c in range(N_CORES):
        sl = slice(c * B_LOCAL, (c + 1) * B_LOCAL)
        in_maps.append({
            "xr": actT(x_re, sl), "xi": actT(x_im, sl), "xs": actT(x_s, sl),
            "hr": actT(h_re, sl), "hi": actT(h_im, sl), "hs": actT(h_s, sl),
            "wx": wxn, "wh": whn, "bias": bias,
        })
    return in_maps


def kernel(**inputs):
    if "nc" not in _CACHE:
        nc = _build(split_for_hw=False)
        try:
            from concourse.timeline_sim import TimelineSim
            LAST_RUN_INFO["timeline_ns"] = int(TimelineSim(nc).simulate())
        except Exception:
            pass
        _CACHE["nc"] = _split_waits(nc)
    nc = _CACHE["nc"]

    in_maps = _prep(inputs)
    res = run_bass_kernel_spmd(nc, in_maps, list(range(N_CORES)))
    LAST_RUN_INFO["exec_time_ns"] = res.exec_time_ns

    out = np.empty((B_FULL, 512, 2), np.float32)
    for c, r in enumerate(res.results):
        sl = slice(c * B_LOCAL, (c + 1) * B_LOCAL)
        out[sl, :, 0] = r["outr"].T
        out[sl, :, 1] = r["outi"].T
    return out

